# revision 1
# baseline (speedup 1.0000x reference)
"""Distributed CLIP loss on 8 Trainium2 NeuronCores (Bass/Tile).

Strategy (data-parallel over image rows, per the distributed-CLIP pattern):
  - Core i owns image rows [2048*i, 2048*(i+1)).  It receives its image shard
    transposed (d-major, bf16) plus the FULL text matrix transposed and
    *rolled* by -2048*i rows, so the diagonal block of the logits always
    lands in local columns [0, 2048) — every core runs the identical program.
  - On device, each core computes its (2048 x 16384) block of
    E = exp(scale * img @ txt^T + bias) tile-by-tile (PE matmul in bf16,
    fp32 PSUM accumulation over the 768-dim contraction; ScalarE exp) and
    reduces it on the fly:
      * row sums of E    (fused into the exp op's accum_out)   -> zrow
      * row max  of E    (VectorE reduce)                      -> rowmax
      * diagonal of E    (VectorE masked reduce with identity) -> diag
      * per-partition column sums/maxes over the 16 row-tiles  -> colsum/colmax
  - The host finishes the job: partition/core reductions of colsum/colmax,
    log-sum-exp assembly, the two CE means, and the argmax==label accuracies
    via (max == diag) equality in E-domain (exp is monotone; all values are
    produced by the same device computation, so equality is bit-faithful).

Since both feature matrices are L2-normalized, |logits| <= scale <= 100 only
if scale is small; we subtract a host-computed upper bound when needed so the
exp never overflows, and the shift cancels in the loss.
"""

import math

import ml_dtypes
import numpy as np

import bass_rust
import concourse.bass as bass
import concourse.tile as tile
from concourse import mybir
from concourse.bass_utils import run_bass_kernel_spmd
from concourse.vector_clock import ScopedClock

N_CORES = 8
B = 16384
D = 768
BL = B // N_CORES          # 2048 local image rows per core
N_RT = BL // 128           # 16 row tiles of 128 rows
N_G = B // 2048            # 8 column groups of 2048 columns
N_KK = D // 128            # 6 contraction chunks
BF16 = mybir.dt.bfloat16
F32 = mybir.dt.float32

_MAXW = 1  # this walrus build allows a single sync-wait per CTRL instruction
_SKIP_DVE_STATS = False  # debug/benchmark knob: drop col/row stat DVE ops
_GROUP_W = 2048  # column-group width (2048: 4 PSUM banks double-buffered;
                 #                      4096: all 8 banks single-buffered)


def _patched_drain_and_barrier(self, tick_clock, wait_clock):
    """Tail drain with its waits split one-per-instruction (walrus limit)."""
    nc = self.nc
    drain_inst = nc.sync.drain()
    wait_clock.add_sem_waits(
        drain_inst.ins, ScopedClock({None: tick_clock.global_clock})
    )
    si = drain_inst.ins.sync_info
    waits = list(si.on_wait or [])
    if len(waits) > _MAXW:
        si.on_wait = waits[:_MAXW]
        rest = waits[_MAXW:]
        for i in range(0, len(rest), _MAXW):
            extra = nc.sync.drain()
            extra.ins.sync_info = bass_rust.SyncInfo(
                on_wait=rest[i : i + _MAXW], on_update=[]
            )
    nc.all_engine_barrier()
    assert self.sems is not None
    popped = nc._tile_sem_poison_stack.pop()
    assert popped is self._sem_poison
    nc.clear_and_free_semaphores(list(self.sems.allocated().values()))
    nc.all_engine_barrier()


tile.TileContext._drain_and_barrier = _patched_drain_and_barrier

_orig_lower_ordered_insts = tile.TileContext._lower_ordered_insts


def _patched_lower_ordered_insts(self, ordered):
    """Split multi-wait instructions: this walrus build allows one sync-wait
    per ISA instruction, so carry the extras on same-engine NOPs in front."""
    nc = self.nc
    for bb_name, insts in ordered.items():
        new_insts = []
        for inst in insts:
            si = inst.sync_info
            if (
                si is not None
                and si.on_wait
                and len(si.on_wait) > _MAXW
                and inst.engine != mybir.EngineType.Unassigned
            ):
                waits = list(si.on_wait)
                si.on_wait = waits[-_MAXW:]
                carry = waits[: -_MAXW]
                for i in range(0, len(carry), _MAXW):
                    nop = mybir.InstNoOp(
                        name=nc.get_next_instruction_name(),
                        engine=inst.engine,
                        ins=[],
                        outs=[],
                        sync_info=bass_rust.SyncInfo(
                            on_wait=carry[i : i + _MAXW], on_update=[]
                        ),
                    )
                    new_insts.append(nop)
            new_insts.append(inst)
        ordered[bb_name] = new_insts
    return _orig_lower_ordered_insts(self, ordered)


tile.TileContext._lower_ordered_insts = _patched_lower_ordered_insts


def _dedup_ldweights(nc) -> int:
    """Remove back-to-back InstLdweights that reload identical weights.

    tile_legalize pairs every matmul with its own LDWEIGHTS even when 4
    consecutive matmuls share the same stationary tile; the reload costs
    ~13 ns/matmul of exposed PE time.  Removal is safe ONLY because the
    weights tiles here (img_sb) are written once and never overwritten, so
    the PE array state stays valid across the elided reloads.  LDWs carrying
    any sync wait/update are kept (their sem bookkeeping must not change),
    and any other PE instruction resets the tracking.
    """
    removed = 0
    for f in nc.m.functions:
        for bb in f.blocks:
            insts = list(bb.instructions)
            keep = []
            last_key = None
            changed = False
            for ins in insts:
                tn = type(ins).__name__
                if tn == "InstLdweights":
                    si = ins.sync_info
                    clean = si is None or (not si.on_wait and not si.on_update)
                    key = (
                        str(ins.ins[0]),
                        str(ins.is_transpose),
                        str(getattr(ins, "tile_position", None)),
                    )
                    if clean and key == last_key:
                        removed += 1
                        changed = True
                        continue
                    last_key = key
                elif tn == "InstMatmult":
                    pass  # matmuls leave the loaded weights untouched
                elif getattr(ins, "engine", None) == mybir.EngineType.PE:
                    last_key = None  # unknown PE op: stop eliding
                keep.append(ins)
            if changed:
                bb.instructions = keep
    return removed


def build_program(scale: float, bias: float, reps: int = 1) -> bass.Bass:
    """Build the per-core Bass program (identical on all 8 cores).

    reps > 1 repeats the whole computation for slope-based timing (the
    per-launch dispatch overhead here is ~77ms, far above kernel time)."""
    nc = bass.Bass("TRN2", target_bir_lowering=False, debug=False)

    W = _GROUP_W              # column-group width
    n_g = B // W              # number of column groups
    nb = W // 512             # PSUM banks per group

    imgT = nc.dram_tensor("imgT", (D, BL), BF16, kind="ExternalInput").ap()
    txtT = nc.dram_tensor("txtT", (D, B), BF16, kind="ExternalInput").ap()
    ident = nc.dram_tensor("ident", (128, 128), F32, kind="ExternalInput").ap()

    colsum_d = nc.dram_tensor("colsum", (n_g, 128, W), F32, kind="ExternalOutput").ap()
    colmax_d = nc.dram_tensor("colmax", (n_g, 128, W), F32, kind="ExternalOutput").ap()
    zrow_d = nc.dram_tensor("zrow", (128, N_RT), F32, kind="ExternalOutput").ap()
    rowmax_d = nc.dram_tensor("rowmax", (128, N_RT), F32, kind="ExternalOutput").ap()
    diag_d = nc.dram_tensor("diag", (128, N_RT), F32, kind="ExternalOutput").ap()

    EXP = mybir.ActivationFunctionType.Exp
    X = mybir.AxisListType.X

    ep_bufs = 3 if W == 2048 else 2
    acc_bufs = 2 if W == 2048 else 1

    with tile.TileContext(nc) as tc:
        with tc.tile_pool(name="const", bufs=1) as constp, \
             tc.tile_pool(name="imgp", bufs=1) as imgp, \
             tc.tile_pool(name="txtp", bufs=2) as txtp, \
             tc.tile_pool(name="psum", bufs=8, space="PSUM") as psump, \
             tc.tile_pool(name="ep", bufs=ep_bufs) as ep, \
             tc.tile_pool(name="accs", bufs=acc_bufs) as accp, \
             tc.tile_pool(name="stats", bufs=1) as statp, \
             tc.tile_pool(name="dscr", bufs=2) as dscrp:

            ident_sb = constp.tile([128, 128], F32)
            nc.sync.dma_start(ident_sb[:], ident)

            img_sb = imgp.tile([128, N_KK, BL], BF16)
            for kk in range(N_KK):
                nc.sync.dma_start(
                    img_sb[:, kk, :], imgT[kk * 128 : (kk + 1) * 128, :]
                )

            # per-(rt, g, bank) fused row sums from the exp ops; 32 contiguous
            # slots per rt.  rowmax: n_g contiguous slots per rt.
            rowsum_slots = statp.tile([128, N_RT * 32], F32)
            rowmax_slots = statp.tile([128, N_RT * n_g], F32)
            diag_sb = statp.tile([128, N_RT], F32)
            zrow_sb = statp.tile([128, N_RT], F32)
            rowmax_sb = statp.tile([128, N_RT], F32)

            if _SKIP_DVE_STATS:
                nc.gpsimd.memset(rowmax_slots[:], 0.0)
            for rep in range(reps):
              for g in range(n_g):
                txt_g = txtp.tile([128, N_KK, W], BF16, tag="txt_g", name=f"txt_{rep}_{g}")
                for kk in range(N_KK):
                    nc.sync.dma_start(
                        txt_g[:, kk, :],
                        txtT[kk * 128 : (kk + 1) * 128, g * W : (g + 1) * W],
                    )
                colsum_acc = accp.tile([128, W], F32, tag="cs")
                colmax_acc = accp.tile([128, W], F32, tag="cm")
                if _SKIP_DVE_STATS:
                    nc.gpsimd.memset(colsum_acc[:], 0.0)
                    nc.gpsimd.memset(colmax_acc[:], 0.0)
                for rt in range(N_RT):
                    pb = [
                        psump.tile([128, 512], F32, tag="pb", name=f"pb{g}_{rt}_{b}")
                        for b in range(nb)
                    ]
                    lhsT = img_sb[:, :, rt * 128 : (rt + 1) * 128]
                    for kk in range(N_KK):
                        for b in range(nb):
                            nc.tensor.matmul(
                                pb[b][:],
                                lhsT[:, kk, :],
                                txt_g[:, kk, b * 512 : (b + 1) * 512],
                                start=(kk == 0),
                                stop=(kk == N_KK - 1),
                            )
                    e_t = ep.tile([128, W], F32, tag="e")
                    for b in range(nb):
                        s = rt * 32 + g * nb + b
                        nc.scalar.activation(
                            out=e_t[:, b * 512 : (b + 1) * 512],
                            in_=pb[b][:],
                            func=EXP,
                            scale=scale,
                            bias=bias,
                            accum_out=rowsum_slots[:, s : s + 1],
                        )
                    if not _SKIP_DVE_STATS:
                        if rt == 0:
                            nc.vector.tensor_copy(colsum_acc[:], e_t[:])
                            nc.vector.tensor_copy(colmax_acc[:], e_t[:])
                        else:
                            nc.vector.tensor_add(colsum_acc[:], colsum_acc[:], e_t[:])
                            nc.vector.tensor_max(colmax_acc[:], colmax_acc[:], e_t[:])
                        s = rt * n_g + g
                        nc.vector.reduce_max(
                            out=rowmax_slots[:, s : s + 1], in_=e_t[:], axis=X
                        )
                    if g == 0:
                        scr = dscrp.tile([128, 128], F32, tag="scr")
                        nc.vector.tensor_mul(
                            scr[:], e_t[:, rt * 128 : (rt + 1) * 128], ident_sb[:]
                        )
                        nc.vector.reduce_max(
                            out=diag_sb[:, rt : rt + 1], in_=scr[:], axis=X
                        )
                nc.sync.dma_start(colsum_d[g], colsum_acc[:])
                nc.sync.dma_start(colmax_d[g], colmax_acc[:])

            for rt in range(N_RT):
                nc.vector.reduce_sum(
                    out=zrow_sb[:, rt : rt + 1],
                    in_=rowsum_slots[:, rt * 32 : (rt + 1) * 32],
                    axis=X,
                )
                nc.vector.reduce_max(
                    out=rowmax_sb[:, rt : rt + 1],
                    in_=rowmax_slots[:, rt * n_g : (rt + 1) * n_g],
                    axis=X,
                )
            nc.sync.dma_start(zrow_d, zrow_sb[:])
            nc.sync.dma_start(rowmax_d, rowmax_sb[:])
            nc.sync.dma_start(diag_d, diag_sb[:])

    _dedup_ldweights(nc)
    return nc


def prepare_inputs(image_features, text_features):
    """Host-side sharding: bf16 cast, transposes, per-core text roll."""
    img = np.ascontiguousarray(np.asarray(image_features, dtype=np.float32))
    txt = np.ascontiguousarray(np.asarray(text_features, dtype=np.float32))
    img_bf = img.astype(ml_dtypes.bfloat16)
    txt_bf = txt.astype(ml_dtypes.bfloat16)
    imgT_full = np.ascontiguousarray(img_bf.T)      # (D, B)
    txtT_full = np.ascontiguousarray(txt_bf.T)      # (D, B)
    ident = np.eye(128, dtype=np.float32)
    in_maps = []
    for i in range(N_CORES):
        imgT_i = np.ascontiguousarray(imgT_full[:, i * BL : (i + 1) * BL])
        txtT_i = np.roll(txtT_full, -BL * i, axis=1)
        in_maps.append({"imgT": imgT_i, "txtT": txtT_i, "ident": ident})
    return in_maps


def postprocess(results, scale_unused=None):
    """Host-side gather/reduce of the per-core stats -> (loss, accs)."""
    zrow = np.empty(B, dtype=np.float64)
    rowmax = np.empty(B, dtype=np.float64)
    diag = np.empty(B, dtype=np.float64)
    zcol = np.zeros(B, dtype=np.float64)
    colmax = np.full(B, -np.inf, dtype=np.float64)
    for i, r in enumerate(results):
        # (128, 16) -> local row index 128*rt + p
        zrow[i * BL : (i + 1) * BL] = r["zrow"].T.reshape(-1).astype(np.float64)
        rowmax[i * BL : (i + 1) * BL] = r["rowmax"].T.reshape(-1).astype(np.float64)
        diag[i * BL : (i + 1) * BL] = r["diag"].T.reshape(-1).astype(np.float64)
        # (8, 128, 2048): local (rolled) col 2048*g + c; partial over partitions
        cs = r["colsum"].astype(np.float64).sum(axis=1).reshape(-1)
        cm = r["colmax"].astype(np.float64).max(axis=1).reshape(-1)
        # local col 0 corresponds to global col 2048*i (text was rolled by -2048*i)
        zcol += np.roll(cs, BL * i)
        colmax = np.maximum(colmax, np.roll(cm, BL * i))

    loss_i2t = np.mean(np.log(zrow) - np.log(diag))
    loss_t2i = np.mean(np.log(zcol) - np.log(diag))
    loss = (loss_i2t + loss_t2i) / 2.0
    i2t_acc = np.mean(rowmax == diag)
    t2i_acc = np.mean(colmax == diag)
    return (
        np.float32(loss),
        np.float32(i2t_acc),
        np.float32(t2i_acc),
    )


_program_cache: dict[tuple[float, float], bass.Bass] = {}


def get_program(scale: float, bias: float) -> bass.Bass:
    key = (scale, bias)
    if key not in _program_cache:
        _program_cache[key] = build_program(scale, bias)
    return _program_cache[key]


def compute_scale_bias(image_features, text_features, logit_scale):
    ls = float(np.asarray(logit_scale))
    scale = 100.0 if ls >= math.log(100.0) else float(math.exp(ls))
    # |logits| <= scale * max|img_i| * max|txt_j|; keep exp argument <= ~70
    # so f32 never overflows even for unnormalized inputs.
    img = np.asarray(image_features, dtype=np.float32)
    txt = np.asarray(text_features, dtype=np.float32)
    ni = float(np.sqrt((img.astype(np.float64) ** 2).sum(axis=1).max()))
    nt = float(np.sqrt((txt.astype(np.float64) ** 2).sum(axis=1).max()))
    bound = scale * ni * nt
    bias = -max(0.0, bound - 70.0)
    return scale, bias


def kernel(image_features, text_features, logit_scale):
    scale, bias = compute_scale_bias(image_features, text_features, logit_scale)
    nc = get_program(scale, bias)
    in_maps = prepare_inputs(image_features, text_features)
    try:
        res = run_bass_kernel_spmd(nc, in_maps, core_ids=list(range(N_CORES)))
    except Exception:
        # transient accelerator hiccups have been observed on this relay;
        # one retry on a fresh attempt usually clears them
        import time as _time

        _time.sleep(2.0)
        res = run_bass_kernel_spmd(nc, in_maps, core_ids=list(range(N_CORES)))
    return postprocess(res.results)



# revision 5
# speedup vs baseline: 2729.6729x; 2729.6729x over previous
"""Distributed CLIP loss on 8 Trainium2 NeuronCores (Bass/Tile).

Strategy (data-parallel over image rows, per the distributed-CLIP pattern):
  - Core i owns image rows [2048*i, 2048*(i+1)).  It receives its image shard
    transposed (d-major, fp8e4, pre-scaled x64) plus the FULL text matrix
    transposed and *rolled* by -2048*i rows, so the diagonal block of the
    logits always lands in local columns [0, 2048) - every core runs the
    identical program.
  - On device, each core computes its (2048 x 16384) block of
    E = exp(scale' * img8 @ txt8^T) tile-by-tile:
      * PE: fp8 DoubleRow matmuls (contraction 768 = 3 passes of 2x128),
        fp32 PSUM accumulation, one [128, 2048] 4-bank tile per (rt, g).
      * ACT: one wide exp per (rt, g) reading all 4 banks, writing bf16
        e_t to SBUF, with the f32 row-sum fused via accum_out.
      * DVE (bf16 2x mode): column-sum and column-max accumulators per
        group, row-max accumulator per row-tile (the last group's update is
        a tensor_tensor_reduce whose accum emits the final row max), and
        the diagonal extracted via mult-by-identity + max-reduce.
      * Pool/GpSimd: column-max for the top groups, offloaded to balance
        engine occupancy.
  - The host finishes: partition/core reductions of colsum/colmax,
    log-sum-exp assembly, the two CE means, and the argmax==label accuracies
    via (max == diag) equality in E-domain (exp is monotone; all values are
    produced by the same device computation, so equality is bit-faithful).

Both feature matrices are L2-normalized so |logits| <= scale <= 100; the
fp8 inputs are pre-scaled by S=64 to use the e4m3 range (max |elem| ~13,
well under the TRN e4m3 limit of 240) and the activation scale divides the
S^2 back out.  A host-computed negative bias guards exp overflow for
unnormalized inputs; the shift cancels in the loss.
"""

import math

import ml_dtypes
import numpy as np

import bass_rust
import concourse.bass as bass
import concourse.tile as tile
from concourse import mybir
from concourse.bass_utils import run_bass_kernel_spmd
from concourse.vector_clock import ScopedClock

N_CORES = 8
B = 16384
D = 768
BL = B // N_CORES          # 2048 local image rows per core
N_RT = BL // 128           # 16 row tiles of 128 rows
W = 2048                   # column-group width (4 PSUM banks, double-buffered)
N_G = B // W               # 8 column groups
N_KK = D // 128            # 6 contraction chunks of 128
N_J = N_KK // 2            # 3 DoubleRow passes (2 chunks each)
NB = W // 512              # 4 PSUM banks per group
S_FP8 = 64.0               # fp8 pre-scale applied to both operands on host

FP8 = mybir.dt.float8e4
BF16 = mybir.dt.bfloat16
F32 = mybir.dt.float32

# colsum accumulation is split between DVE and Pool/GpSimd (the only
# tensor_tensor op this walrus build supports on Pool is add): DVE owns
# row-tiles [0, POOL_CS_RT) per group, Pool owns the rest, and one DVE add
# merges the two partial accumulators per group.
POOL_CS_RT = 5

_MAXW = 1  # this walrus build allows a single sync-wait per CTRL instruction


def _patched_drain_and_barrier(self, tick_clock, wait_clock):
    """Tail drain with its waits split one-per-instruction (walrus limit)."""
    nc = self.nc
    drain_inst = nc.sync.drain()
    wait_clock.add_sem_waits(
        drain_inst.ins, ScopedClock({None: tick_clock.global_clock})
    )
    si = drain_inst.ins.sync_info
    waits = list(si.on_wait or [])
    if len(waits) > _MAXW:
        si.on_wait = waits[:_MAXW]
        rest = waits[_MAXW:]
        for i in range(0, len(rest), _MAXW):
            extra = nc.sync.drain()
            extra.ins.sync_info = bass_rust.SyncInfo(
                on_wait=rest[i : i + _MAXW], on_update=[]
            )
    nc.all_engine_barrier()
    assert self.sems is not None
    popped = nc._tile_sem_poison_stack.pop()
    assert popped is self._sem_poison
    nc.clear_and_free_semaphores(list(self.sems.allocated().values()))
    nc.all_engine_barrier()


tile.TileContext._drain_and_barrier = _patched_drain_and_barrier

_orig_lower_ordered_insts = tile.TileContext._lower_ordered_insts


def _patched_lower_ordered_insts(self, ordered):
    """Split multi-wait instructions: this walrus build allows one sync-wait
    per ISA instruction, so carry the extras on same-engine NOPs in front."""
    nc = self.nc
    for bb_name, insts in ordered.items():
        new_insts = []
        for inst in insts:
            si = inst.sync_info
            if (
                si is not None
                and si.on_wait
                and len(si.on_wait) > _MAXW
                and inst.engine != mybir.EngineType.Unassigned
            ):
                waits = list(si.on_wait)
                si.on_wait = waits[-_MAXW:]
                carry = waits[: -_MAXW]
                for i in range(0, len(carry), _MAXW):
                    nop = mybir.InstNoOp(
                        name=nc.get_next_instruction_name(),
                        engine=inst.engine,
                        ins=[],
                        outs=[],
                        sync_info=bass_rust.SyncInfo(
                            on_wait=carry[i : i + _MAXW], on_update=[]
                        ),
                    )
                    new_insts.append(nop)
            new_insts.append(inst)
        ordered[bb_name] = new_insts
    return _orig_lower_ordered_insts(self, ordered)


tile.TileContext._lower_ordered_insts = _patched_lower_ordered_insts


def _dedup_ldweights(nc) -> int:
    """Remove back-to-back InstLdweights that reload identical weights.

    tile_legalize pairs every matmul with its own LDWEIGHTS even when 4
    consecutive matmuls share the same stationary tile; the reload costs
    exposed PE time.  Removal is safe ONLY because the weights tiles here
    (img_sb) are written once and never overwritten, so the PE array state
    stays valid across the elided reloads.  LDWs carrying any sync
    wait/update are kept (their sem bookkeeping must not change), and any
    other PE instruction resets the tracking.
    """
    removed = 0
    for f in nc.m.functions:
        for bb in f.blocks:
            insts = list(bb.instructions)
            keep = []
            last_key = None
            changed = False
            for ins in insts:
                tn = type(ins).__name__
                if tn == "InstLdweights":
                    si = ins.sync_info
                    clean = si is None or (not si.on_wait and not si.on_update)
                    key = (
                        str(ins.ins[0]),
                        str(ins.is_transpose),
                        str(getattr(ins, "perf_mode", None)),
                        str(getattr(ins, "tile_position", None)),
                    )
                    if clean and key == last_key:
                        removed += 1
                        changed = True
                        continue
                    last_key = key
                elif tn == "InstMatmult":
                    pass  # matmuls leave the loaded weights untouched
                elif getattr(ins, "engine", None) == mybir.EngineType.PE:
                    last_key = None  # unknown PE op: stop eliding
                keep.append(ins)
            if changed:
                bb.instructions = keep
    return removed


def build_program(scale: float, bias: float, reps: int = 1) -> bass.Bass:
    """Build the per-core Bass program (identical on all 8 cores).

    reps > 1 repeats the whole computation for slope-based timing (the
    per-launch dispatch overhead here is ~80ms, far above kernel time)."""
    nc = bass.Bass("TRN2", target_bir_lowering=False, debug=False)

    act_scale = scale / (S_FP8 * S_FP8)

    imgT = nc.dram_tensor("imgT", (D, BL), FP8, kind="ExternalInput").ap()
    txtT = nc.dram_tensor("txtT", (D, B), FP8, kind="ExternalInput").ap()
    ident = nc.dram_tensor("ident", (128, 128), BF16, kind="ExternalInput").ap()

    colsum_d = nc.dram_tensor("colsum", (N_G, 128, W), BF16, kind="ExternalOutput").ap()
    colmax_d = nc.dram_tensor("colmax", (N_G, 128, W), BF16, kind="ExternalOutput").ap()
    zrow_d = nc.dram_tensor("zrow", (128, N_RT), F32, kind="ExternalOutput").ap()
    rowmax_d = nc.dram_tensor("rowmax", (128, N_RT), F32, kind="ExternalOutput").ap()
    diag_d = nc.dram_tensor("diag", (128, N_RT), F32, kind="ExternalOutput").ap()

    EXP = mybir.ActivationFunctionType.Exp
    X = mybir.AxisListType.X
    MAX = mybir.AluOpType.max
    MULT = mybir.AluOpType.mult
    DR = mybir.MatmulPerfMode.DoubleRow

    with tile.TileContext(nc) as tc:
        with tc.tile_pool(name="const", bufs=1) as constp, \
             tc.tile_pool(name="imgp", bufs=1) as imgp, \
             tc.tile_pool(name="txtp", bufs=2) as txtp, \
             tc.tile_pool(name="psum", bufs=2, space="PSUM") as psump, \
             tc.tile_pool(name="ep", bufs=3) as ep, \
             tc.tile_pool(name="accs", bufs=2) as accp, \
             tc.tile_pool(name="rowp", bufs=1) as rowp, \
             tc.tile_pool(name="stats", bufs=1) as statp, \
             tc.tile_pool(name="dscr", bufs=2) as dscrp:

            ident_sb = constp.tile([128, 128], BF16)
            nc.sync.dma_start(ident_sb[:], ident)
            bias_sb = constp.tile([128, 1], F32)
            nc.gpsimd.memset(bias_sb[:], float(bias))

            img_sb = imgp.tile([128, N_KK, BL], FP8)
            for kk in range(N_KK):
                nc.sync.dma_start(
                    img_sb[:, kk, :], imgT[kk * 128 : (kk + 1) * 128, :]
                )

            # one f32 row-sum accum slot per (rt, g); zrow = sum over g
            rowsum_slots = statp.tile([128, N_RT * N_G], F32)
            diag_sb = statp.tile([128, N_RT], F32)
            zrow_sb = statp.tile([128, N_RT], F32)
            rowmax_sb = statp.tile([128, N_RT], F32)
            # per-row-tile running max over groups (bf16, 2x DVE mode)
            rowacc = rowp.tile([128, N_RT, W], BF16)

            for rep in range(reps):
              for g in range(N_G):
                txt_g = txtp.tile([128, N_KK, W], FP8, tag="txt_g",
                                  name=f"txt_{rep}_{g}")
                for kk in range(N_KK):
                    nc.sync.dma_start(
                        txt_g[:, kk, :],
                        txtT[kk * 128 : (kk + 1) * 128, g * W : (g + 1) * W],
                    )
                cs_dve = accp.tile([128, W], BF16, tag="cs_d")
                cs_pool = accp.tile([128, W], BF16, tag="cs_p")
                colmax_acc = accp.tile([128, W], BF16, tag="cm")
                for rt in range(N_RT):
                    pb = psump.tile([128, W], F32, tag="pb", name=f"pb{rep}_{g}_{rt}")
                    for j in range(N_J):
                        lhsT = img_sb[:, 2 * j : 2 * j + 2, rt * 128 : (rt + 1) * 128]
                        for b in range(NB):
                            nc.tensor.matmul(
                                pb[:, b * 512 : (b + 1) * 512],
                                lhsT,
                                txt_g[:, 2 * j : 2 * j + 2, b * 512 : (b + 1) * 512],
                                start=(j == 0),
                                stop=(j == N_J - 1),
                                perf_mode=DR,
                            )
                    e_t = ep.tile([128, W], BF16, tag="e")
                    s = rt * N_G + g
                    nc.scalar.activation(
                        out=e_t[:],
                        in_=pb[:],
                        func=EXP,
                        scale=act_scale,
                        bias=bias_sb[:],
                        accum_out=rowsum_slots[:, s : s + 1],
                    )
                    # column sum: DVE owns rt < POOL_CS_RT, Pool the rest
                    if rt == 0:
                        nc.vector.tensor_copy(cs_dve[:], e_t[:])
                    elif rt < POOL_CS_RT:
                        nc.vector.tensor_add(cs_dve[:], cs_dve[:], e_t[:])
                    elif rt == POOL_CS_RT:
                        nc.gpsimd.tensor_copy(cs_pool[:], e_t[:])
                    else:
                        nc.gpsimd.tensor_add(cs_pool[:], cs_pool[:], e_t[:])
                    # column max (DVE; Pool has no max op in this build)
                    if rt == 0:
                        nc.vector.tensor_copy(colmax_acc[:], e_t[:])
                    else:
                        nc.vector.tensor_max(colmax_acc[:], colmax_acc[:], e_t[:])
                    # row max (accumulate over g per row tile)
                    ra = rowacc[:, rt, :]
                    if g == 0:
                        nc.vector.tensor_copy(ra, e_t[:])
                        scr = dscrp.tile([128, 128], BF16, tag="scr",
                                         name=f"scr_{rep}_{rt}")
                        nc.vector.tensor_mul(
                            scr[:], e_t[:, rt * 128 : (rt + 1) * 128], ident_sb[:]
                        )
                        nc.vector.reduce_max(
                            out=diag_sb[:, rt : rt + 1], in_=scr[:], axis=X
                        )
                    else:
                        nc.vector.tensor_max(ra, ra, e_t[:])
                        if g == N_G - 1:
                            nc.vector.reduce_max(
                                out=rowmax_sb[:, rt : rt + 1], in_=ra, axis=X
                            )
                nc.vector.tensor_add(cs_dve[:], cs_dve[:], cs_pool[:])
                nc.sync.dma_start(colsum_d[g], cs_dve[:])
                nc.sync.dma_start(colmax_d[g], colmax_acc[:])

              for rt in range(N_RT):
                nc.vector.reduce_sum(
                    out=zrow_sb[:, rt : rt + 1],
                    in_=rowsum_slots[:, rt * N_G : (rt + 1) * N_G],
                    axis=X,
                )
            nc.sync.dma_start(zrow_d, zrow_sb[:])
            nc.sync.dma_start(rowmax_d, rowmax_sb[:])
            nc.sync.dma_start(diag_d, diag_sb[:])

    _dedup_ldweights(nc)
    return nc


def prepare_inputs(image_features, text_features):
    """Host-side sharding: x64 pre-scale, fp8e4 cast, transposes, text roll."""
    img = np.asarray(image_features, dtype=np.float32) * S_FP8
    txt = np.asarray(text_features, dtype=np.float32) * S_FP8
    # TRN fp8e4 tops out at +-240 (ml_dtypes.float8_e4m3 matches)
    np.clip(img, -240.0, 240.0, out=img)
    np.clip(txt, -240.0, 240.0, out=txt)
    img8 = img.astype(ml_dtypes.float8_e4m3)
    txt8 = txt.astype(ml_dtypes.float8_e4m3)
    imgT_full = np.ascontiguousarray(img8.T)      # (D, B)
    txtT_full = np.ascontiguousarray(txt8.T)      # (D, B)
    ident = np.eye(128, dtype=np.float32).astype(ml_dtypes.bfloat16)
    in_maps = []
    for i in range(N_CORES):
        imgT_i = np.ascontiguousarray(imgT_full[:, i * BL : (i + 1) * BL])
        txtT_i = np.roll(txtT_full, -BL * i, axis=1)
        in_maps.append({"imgT": imgT_i, "txtT": txtT_i, "ident": ident})
    return in_maps


def postprocess(results, scale_unused=None):
    """Host-side gather/reduce of the per-core stats -> (loss, accs)."""
    zrow = np.empty(B, dtype=np.float64)
    rowmax = np.empty(B, dtype=np.float64)
    diag = np.empty(B, dtype=np.float64)
    zcol = np.zeros(B, dtype=np.float64)
    colmax = np.full(B, -np.inf, dtype=np.float64)
    for i, r in enumerate(results):
        # (128, 16) -> local row index 128*rt + p
        zrow[i * BL : (i + 1) * BL] = r["zrow"].T.reshape(-1).astype(np.float64)
        rowmax[i * BL : (i + 1) * BL] = r["rowmax"].T.reshape(-1).astype(np.float64)
        diag[i * BL : (i + 1) * BL] = r["diag"].T.reshape(-1).astype(np.float64)
        # (8, 128, 2048): local (rolled) col 2048*g + c; partial over partitions
        cs = r["colsum"].astype(np.float64).sum(axis=1).reshape(-1)
        cm = r["colmax"].astype(np.float64).max(axis=1).reshape(-1)
        # local col 0 corresponds to global col 2048*i (text was rolled by -2048*i)
        zcol += np.roll(cs, BL * i)
        colmax = np.maximum(colmax, np.roll(cm, BL * i))

    loss_i2t = np.mean(np.log(zrow) - np.log(diag))
    loss_t2i = np.mean(np.log(zcol) - np.log(diag))
    loss = (loss_i2t + loss_t2i) / 2.0
    i2t_acc = np.mean(rowmax == diag)
    t2i_acc = np.mean(colmax == diag)
    return (
        np.float32(loss),
        np.float32(i2t_acc),
        np.float32(t2i_acc),
    )


_program_cache: dict[tuple[float, float], bass.Bass] = {}


def get_program(scale: float, bias: float) -> bass.Bass:
    key = (scale, bias)
    if key not in _program_cache:
        _program_cache[key] = build_program(scale, bias)
    return _program_cache[key]


def compute_scale_bias(image_features, text_features, logit_scale):
    ls = float(np.asarray(logit_scale))
    scale = 100.0 if ls >= math.log(100.0) else float(math.exp(ls))
    # |logits| <= scale * max|img_i| * max|txt_j|; keep exp argument <= ~70
    # so f32 never overflows even for unnormalized inputs.
    img = np.asarray(image_features, dtype=np.float32)
    txt = np.asarray(text_features, dtype=np.float32)
    ni = float(np.sqrt((img.astype(np.float64) ** 2).sum(axis=1).max()))
    nt = float(np.sqrt((txt.astype(np.float64) ** 2).sum(axis=1).max()))
    bound = scale * ni * nt
    bias = -max(0.0, bound * 1.05 - 70.0)
    return scale, bias


def kernel(image_features, text_features, logit_scale):
    scale, bias = compute_scale_bias(image_features, text_features, logit_scale)
    nc = get_program(scale, bias)
    in_maps = prepare_inputs(image_features, text_features)
    try:
        res = run_bass_kernel_spmd(nc, in_maps, core_ids=list(range(N_CORES)))
    except Exception:
        # transient accelerator hiccups have been observed on this relay;
        # one retry on a fresh attempt usually clears them
        import time as _time

        _time.sleep(2.0)
        res = run_bass_kernel_spmd(nc, in_maps, core_ids=list(range(N_CORES)))
    return postprocess(res.results)
